# revision 7
# baseline (speedup 1.0000x reference)
"""Trainium2 Bass kernel for nn_Discriminator_15668040696127.

Computes:
    q, a, d = samples[:, 0], samples[:, 1], samples[:, 2]        # [B, D]
    cos1 = <q,d> / max(||q||*||d||, 1e-6)                         # [B]
    cos2 = <a,d> / max(||a||*||d||, 1e-6)                         # [B]
    score = cos1 @ D_v1 + cos2 @ D_v2                             # scalar
    out = BCE_with_logits(score, labels[0])                       # scalar

Sharding: data-parallel over B across 8 NeuronCores (1024 samples
each).  Each core streams its 48 MiB slice of `samples` and emits the
per-sample weighted-cosine contributions (cos1*w1, cos2*w2) as a
[128, 16] tile; the host gathers the 8 partial tiles, sums them to the
scalar score, and applies the 13-flop BCE epilogue.  No on-device
collective: a tail all-reduce couples every core's measured span to the
slowest/earliest core (NEFF start skew of ~20 us goes straight into the
max-core exec time) and costs another ~14 us of collective latency.

DMA layout: tiles 0..6 are loaded as single [128, 3*4096] contiguous
slabs (per-partition rows of 48 KiB — three 16 KiB strided component
loads are descriptor-rate-limited and run ~15% below HBM roofline).
The last tile's d/q components are hoisted to the head of the queue
(their dd/qd/qq reductions run in the loop warm-up window) and its `a`
component streams last, split in halves, so only ~3 us of a-dependent
work remains after the 48 MiB stream ends.
"""

import os
import sys

import numpy as np

for _p in ("/opt/trn_rl_repo", "/root/.axon_site/_ro/trn_rl_repo"):
    if os.path.isdir(_p) and _p not in sys.path:
        sys.path.append(_p)

import concourse.bass as bass
import concourse.bacc as bacc
import concourse.mybir as mybir
import concourse.tile as tile
from concourse import bass_utils

N_CORES = 8
B, D = 8192, 4096
BS = B // N_CORES          # 1024 samples per core
P = 128                    # SBUF partitions
T = BS // P                # 8 tiles of 128 samples per core
W = 3 * D                  # flattened (q|a|d) row width
EPS = 1e-6

f32 = mybir.dt.float32
Alu = mybir.AluOpType
Act = mybir.ActivationFunctionType

_CACHE = {}


def _build_program():
    nc = bacc.Bacc(
        "TRN2",
        target_bir_lowering=False,
        debug=False,
        num_devices=1,
    )

    samples = nc.dram_tensor("samples", [BS, 3, D], f32, kind="ExternalInput")
    dv1 = nc.dram_tensor("dv1", [BS], f32, kind="ExternalInput")
    dv2 = nc.dram_tensor("dv2", [BS], f32, kind="ExternalInput")
    out = nc.dram_tensor("out", [P, 2 * T], f32, kind="ExternalOutput")

    flat = samples[:].rearrange("b c d -> b (c d)")  # [BS, 12288] contiguous

    with tile.TileContext(nc) as tc:
        with (
            tc.tile_pool(name="data", bufs=2) as data_pool,
            tc.tile_pool(name="junk", bufs=1) as junk_pool,
            tc.tile_pool(name="stats", bufs=1) as stats_pool,
        ):
            # Interleaved stats columns: tile t owns columns 2t (q·d /
            # |q||d|) and 2t+1 (a·d / |a||d|), so each tile's epilogue
            # works on a contiguous [P, 2] slice inside the loop.
            dots = stats_pool.tile([P, 2 * T], f32, tag="dots")
            nprod = stats_pool.tile([P, 2 * T], f32, tag="nprod")
            inv = stats_pool.tile([P, 2 * T], f32, tag="inv")
            contrib = stats_pool.tile([P, 2 * T], f32, tag="contrib")

            L = T - 1  # the last tile, handled out of line
            h = D // 2

            # --- Tile L's d/q loads go FIRST in the DMA queue, into
            # dedicated tiles; their dd/qd/qq work is emitted first on
            # each engine so it runs in the loop's warm-up window.
            dL = stats_pool.tile([P, D], f32, tag="dL")
            qL = stats_pool.tile([P, D], f32, tag="qL")
            nc.sync.dma_start(dL[:], samples[bass.ts(L, P), 2, :])
            nc.scalar.dma_start(qL[:], samples[bass.ts(L, P), 0, :])

            # Small weight loads off the big-DMA ring (SWDGE).  dvb
            # column 2t holds D_v1 tile t, column 2t+1 holds D_v2.
            dvb = stats_pool.tile([P, 2 * T], f32, tag="dvb")
            dvb_v = dvb[:].rearrange("p (t g) -> p t g", g=2)
            nc.gpsimd.dma_start(dvb_v[:, :, 0], dv1[:].rearrange("(n p) -> p n", p=P))
            nc.gpsimd.dma_start(dvb_v[:, :, 1], dv2[:].rearrange("(n p) -> p n", p=P))

            ddL = stats_pool.tile([P, 1], f32, tag="ddL")
            jdL = junk_pool.tile([P, D], f32, tag="junk_dve")
            nc.vector.scalar_tensor_tensor(
                out=jdL[:], in0=dL[:], scalar=1.0, in1=dL[:],
                op0=Alu.mult, op1=Alu.mult, accum_out=ddL[:],
            )
            qdL = stats_pool.tile([P, 1], f32, tag="qdL")
            jdL2 = junk_pool.tile([P, D], f32, tag="junk_dve")
            nc.vector.scalar_tensor_tensor(
                out=jdL2[:], in0=qL[:], scalar=1.0, in1=dL[:],
                op0=Alu.mult, op1=Alu.mult, accum_out=qdL[:],
            )
            nc.vector.tensor_copy(dots[:, 2 * L : 2 * L + 1], qdL[:])
            qqL = stats_pool.tile([P, 1], f32, tag="qqL")
            jaL = junk_pool.tile([P, D], f32, tag="junk_act")
            nc.scalar.activation(
                out=jaL[:], in_=qL[:], func=Act.Square, accum_out=qqL[:],
            )
            nc.vector.tensor_mul(nprod[:, 2 * L : 2 * L + 1], qqL[:], ddL[:])
            # Tile L's q-column epilogue runs early too; only the
            # a-column (2L+1) stays on the tail.
            cq = slice(2 * L, 2 * L + 1)
            nc.scalar.activation(inv[:, cq], nprod[:, cq], Act.Ln)
            nc.scalar.activation(inv[:, cq], inv[:, cq], Act.Exp, scale=0.5)
            nc.vector.tensor_scalar_max(inv[:, cq], inv[:, cq], EPS)
            nc.vector.reciprocal(inv[:, cq], inv[:, cq])
            nc.vector.tensor_mul(contrib[:, cq], dots[:, cq], inv[:, cq])
            nc.vector.tensor_mul(contrib[:, cq], contrib[:, cq], dvb[:, cq])

            for t in range(T - 1):
                # One contiguous [128, 12288] slab per tile: 48 KiB
                # per-partition rows keep the 16 HWDGE engines
                # bandwidth-limited instead of descriptor-limited.  The
                # slab is split across both HWDGE trigger queues
                # (SP + Activation) so two descriptor rings feed the DMA
                # engines concurrently and fetch gaps overlap.
                s_t = data_pool.tile([P, W], f32, tag="s")
                hp = P // 2
                r0 = t * P
                nc.sync.dma_start(s_t[0:hp, :], flat[r0 : r0 + hp, :])
                nc.scalar.dma_start(s_t[hp:P, :], flat[r0 + hp : r0 + P, :])
                q = s_t[:, 0:D]
                a = s_t[:, D : 2 * D]
                d = s_t[:, 2 * D : 3 * D]

                # DVE: fused product + per-partition accumulate
                # (scalar_tensor_tensor; accum_out must be a standalone
                # tile — strided accum destinations crash the HW).
                dve_accs = {}
                for src0, src1, col, atag in (
                    (d, d, None, "dd1"),
                    (q, d, 2 * t, "qd1"),
                    (a, d, 2 * t + 1, "ad1"),
                ):
                    jd = junk_pool.tile([P, D], f32, tag="junk_dve")
                    acc = junk_pool.tile([P, 1], f32, tag=atag)
                    nc.vector.scalar_tensor_tensor(
                        out=jd[:], in0=src0, scalar=1.0, in1=src1,
                        op0=Alu.mult, op1=Alu.mult, accum_out=acc[:],
                    )
                    dve_accs[atag] = acc
                    if col is not None:
                        nc.vector.tensor_copy(dots[:, col : col + 1], acc[:])

                # ACT: square + accumulate for the q/a norms, then the
                # norm products (qq*dd, aa*dd) land in this tile's
                # columns.
                for src0, col, atag in ((q, 2 * t, "qq1"), (a, 2 * t + 1, "aa1")):
                    ja = junk_pool.tile([P, D], f32, tag="junk_act")
                    acc = junk_pool.tile([P, 1], f32, tag=atag)
                    nc.scalar.activation(
                        out=ja[:], in_=src0, func=Act.Square, accum_out=acc[:],
                    )
                    nc.vector.tensor_mul(
                        nprod[:, col : col + 1], acc[:], dve_accs["dd1"][:]
                    )

                # Per-tile epilogue on the contiguous [P, 2] slice —
                # hidden under the next tile's DMA.
                # cos = dot / max(sqrt(nprod), EPS), with
                # sqrt(v) = exp(0.5*ln(v)) so the whole kernel stays on
                # the natural_log_exp activation table (no reload).
                c2 = slice(2 * t, 2 * t + 2)
                nc.scalar.activation(inv[:, c2], nprod[:, c2], Act.Ln)
                nc.scalar.activation(inv[:, c2], inv[:, c2], Act.Exp, scale=0.5)
                nc.vector.tensor_scalar_max(inv[:, c2], inv[:, c2], EPS)
                nc.vector.reciprocal(inv[:, c2], inv[:, c2])
                nc.vector.tensor_mul(contrib[:, c2], dots[:, c2], inv[:, c2])
                nc.vector.tensor_mul(contrib[:, c2], contrib[:, c2], dvb[:, c2])

            # --- Tile L's a arrives last (split in half); only <a,d>,
            # |a|^2 and the final [P,1] epilogue remain on the tail.
            aL = stats_pool.tile([P, D], f32, tag="aL")
            nc.sync.dma_start(aL[:, 0:h], samples[bass.ts(L, P), 1, 0:h])
            nc.scalar.dma_start(aL[:, h:D], samples[bass.ts(L, P), 1, h:D])

            # Everything but the last tile's a-column is final here; dump
            # it early so only 512 B of output DMA remains on the tail.
            nc.sync.dma_start(out[:, 0 : 2 * T - 1], contrib[:, 0 : 2 * T - 1])

            jd = junk_pool.tile([P, D], f32, tag="junk_dve")
            adA = junk_pool.tile([P, 1], f32, tag="ad1")
            adB = junk_pool.tile([P, 1], f32, tag="ad1b")
            nc.vector.scalar_tensor_tensor(
                out=jd[:, 0:h], in0=aL[:, 0:h], scalar=1.0, in1=dL[:, 0:h],
                op0=Alu.mult, op1=Alu.mult, accum_out=adA[:],
            )
            nc.vector.scalar_tensor_tensor(
                out=jd[:, h:D], in0=aL[:, h:D], scalar=1.0, in1=dL[:, h:D],
                op0=Alu.mult, op1=Alu.mult, accum_out=adB[:],
            )
            acol = 2 * L + 1
            ca = slice(acol, acol + 1)
            nc.vector.tensor_add(dots[:, ca], adA[:], adB[:])

            ja2 = junk_pool.tile([P, D], f32, tag="junk_act")
            aaA = junk_pool.tile([P, 1], f32, tag="aa1")
            aaB = junk_pool.tile([P, 1], f32, tag="aa1b")
            aa_sum = junk_pool.tile([P, 1], f32, tag="aa_sum")
            nc.scalar.activation(
                out=ja2[:, 0:h], in_=aL[:, 0:h], func=Act.Square,
                accum_out=aaA[:],
            )
            nc.scalar.activation(
                out=ja2[:, h:D], in_=aL[:, h:D], func=Act.Square,
                accum_out=aaB[:],
            )
            nc.vector.tensor_add(aa_sum[:], aaA[:], aaB[:])
            nc.vector.tensor_mul(nprod[:, ca], aa_sum[:], ddL[:])

            nc.scalar.activation(inv[:, ca], nprod[:, ca], Act.Ln)
            nc.scalar.activation(inv[:, ca], inv[:, ca], Act.Exp, scale=0.5)
            nc.vector.tensor_scalar_max(inv[:, ca], inv[:, ca], EPS)
            nc.vector.reciprocal(inv[:, ca], inv[:, ca])
            nc.vector.tensor_mul(contrib[:, ca], dots[:, ca], inv[:, ca])
            nc.vector.tensor_mul(contrib[:, ca], contrib[:, ca], dvb[:, ca])

            nc.sync.dma_start(out[:, 2 * T - 1 : 2 * T], contrib[:, ca])

    nc.compile()
    return nc


def _get_program():
    if "nc" not in _CACHE:
        _CACHE["nc"] = _build_program()
    return _CACHE["nc"]


def kernel(samples, labels, D_v1, D_v2):
    samples = np.asarray(samples, dtype=np.float32)
    labels = np.asarray(labels, dtype=np.float32)
    D_v1 = np.asarray(D_v1, dtype=np.float32)
    D_v2 = np.asarray(D_v2, dtype=np.float32)
    assert samples.shape == (B, 3, D), samples.shape

    nc = _get_program()

    in_maps = []
    for c in range(N_CORES):
        sl = slice(c * BS, (c + 1) * BS)
        in_maps.append(
            {
                "samples": np.ascontiguousarray(samples[sl]),
                "dv1": np.ascontiguousarray(D_v1[sl]),
                "dv2": np.ascontiguousarray(D_v2[sl]),
            }
        )

    _tc = os.environ.get("KERNEL_TRACE_CORES")
    _kw = {"trace_cores": [int(x) for x in _tc.split(",")]} if _tc else {}
    try:
        res = bass_utils.run_bass_kernel_spmd(
            nc, in_maps, core_ids=list(range(N_CORES)), **_kw
        )
    except Exception:
        # A previously-wedged NeuronCore surfaces as an unrecoverable
        # exec error on the first attempt; the runtime resets it, so a
        # single retry recovers.
        res = bass_utils.run_bass_kernel_spmd(
            nc, in_maps, core_ids=list(range(N_CORES)), **_kw
        )
    _CACHE["last_results"] = res

    # Gather/unshard: sum the per-core per-sample contributions to the
    # scalar score, then the 13-flop BCE epilogue.
    s = 0.0
    for c in range(N_CORES):
        s += float(np.sum(np.asarray(res.results[c]["out"], dtype=np.float64)))
    y = float(labels.reshape(-1)[0])
    bce = max(s, 0.0) - s * y + np.log1p(np.exp(-abs(s)))
    return np.asarray(bce, dtype=np.float32).reshape(())


# revision 10
# speedup vs baseline: 1.5394x; 1.5394x over previous
"""Trainium2 Bass kernel for nn_Discriminator_15668040696127.

Computes:
    q, a, d = samples[:, 0], samples[:, 1], samples[:, 2]        # [B, D]
    cos1 = <q,d> / max(||q||*||d||, 1e-6)                         # [B]
    cos2 = <a,d> / max(||a||*||d||, 1e-6)                         # [B]
    score = cos1 @ D_v1 + cos2 @ D_v2                             # scalar
    out = BCE_with_logits(score, labels[0])                       # scalar

Sharding: data-parallel over B across 8 NeuronCores (1024 samples
each).  Each core streams its 48 MiB slice of `samples` and emits the
per-sample weighted-cosine contributions (cos1*w1, cos2*w2) as a
[128, 16] tile; the host gathers the 8 partial tiles, sums them to the
scalar score, and applies the 13-flop BCE epilogue.  No on-device
collective: a tail all-reduce couples every core's measured span to the
slowest/earliest core (NEFF start skew of ~20 us goes straight into the
max-core exec time) and costs another ~14 us of collective latency.

DMA layout: tiles 0..6 are loaded as single [128, 3*4096] contiguous
slabs (per-partition rows of 48 KiB — three 16 KiB strided component
loads are descriptor-rate-limited and run ~15% below HBM roofline).
The last tile's d/q components are hoisted to the head of the queue
(their dd/qd/qq reductions run in the loop warm-up window) and its `a`
component streams last, split in halves, so only ~3 us of a-dependent
work remains after the 48 MiB stream ends.
"""

import os
import sys

import numpy as np

for _p in ("/opt/trn_rl_repo", "/root/.axon_site/_ro/trn_rl_repo"):
    if os.path.isdir(_p) and _p not in sys.path:
        sys.path.append(_p)

import concourse.bass as bass
import concourse.bacc as bacc
import concourse.mybir as mybir
import concourse.tile as tile
from concourse import bass_utils

N_CORES = 8
B, D = 8192, 4096
BS = B // N_CORES          # 1024 samples per core
P = 128                    # SBUF partitions
T = BS // P                # 8 tiles of 128 samples per core
W = 3 * D                  # flattened (q|a|d) row width
EPS = 1e-6

f32 = mybir.dt.float32
Alu = mybir.AluOpType
Act = mybir.ActivationFunctionType

_CACHE = {}


def _build_program():
    nc = bacc.Bacc(
        "TRN2",
        target_bir_lowering=False,
        debug=False,
        num_devices=1,
    )

    samples = nc.dram_tensor("samples", [BS, 3, D], f32, kind="ExternalInput")
    dv1 = nc.dram_tensor("dv1", [BS], f32, kind="ExternalInput")
    dv2 = nc.dram_tensor("dv2", [BS], f32, kind="ExternalInput")
    out = nc.dram_tensor("out", [P, 2 * T], f32, kind="ExternalOutput")

    flat = samples[:].rearrange("b c d -> b (c d)")  # [BS, 12288] contiguous

    with tile.TileContext(nc) as tc:
        with (
            tc.tile_pool(name="data", bufs=2) as data_pool,
            tc.tile_pool(name="junk", bufs=1) as junk_pool,
            tc.tile_pool(name="stats", bufs=1) as stats_pool,
        ):
            # Interleaved stats columns: tile t owns columns 2t (q·d /
            # |q||d|) and 2t+1 (a·d / |a||d|), so each tile's epilogue
            # works on a contiguous [P, 2] slice inside the loop.
            dots = stats_pool.tile([P, 2 * T], f32, tag="dots")
            nprod = stats_pool.tile([P, 2 * T], f32, tag="nprod")
            inv = stats_pool.tile([P, 2 * T], f32, tag="inv")
            contrib = stats_pool.tile([P, 2 * T], f32, tag="contrib")

            L = T - 1  # the last tile, handled out of line
            h = D // 2

            # --- Tile L's d/q loads go FIRST in the DMA queue, into
            # dedicated tiles; their dd/qd/qq work is emitted first on
            # each engine so it runs in the loop's warm-up window.
            dL = stats_pool.tile([P, D], f32, tag="dL")
            qL = stats_pool.tile([P, D], f32, tag="qL")
            nc.sync.dma_start(dL[:], samples[bass.ts(L, P), 2, :])
            nc.sync.dma_start(qL[:], samples[bass.ts(L, P), 0, :])

            # Small weight loads off the big-DMA ring (SWDGE).  dvb
            # column 2t holds D_v1 tile t, column 2t+1 holds D_v2.
            dvb = stats_pool.tile([P, 2 * T], f32, tag="dvb")
            dvb_v = dvb[:].rearrange("p (t g) -> p t g", g=2)
            nc.gpsimd.dma_start(dvb_v[:, :, 0], dv1[:].rearrange("(n p) -> p n", p=P))
            nc.gpsimd.dma_start(dvb_v[:, :, 1], dv2[:].rearrange("(n p) -> p n", p=P))

            ddL = stats_pool.tile([P, 1], f32, tag="ddL")
            jdL = junk_pool.tile([P, D], f32, tag="junk_dve")
            nc.vector.scalar_tensor_tensor(
                out=jdL[:], in0=dL[:], scalar=1.0, in1=dL[:],
                op0=Alu.mult, op1=Alu.mult, accum_out=ddL[:],
            )
            qdL = stats_pool.tile([P, 1], f32, tag="qdL")
            jdL2 = junk_pool.tile([P, D], f32, tag="junk_dve")
            nc.vector.scalar_tensor_tensor(
                out=jdL2[:], in0=qL[:], scalar=1.0, in1=dL[:],
                op0=Alu.mult, op1=Alu.mult, accum_out=qdL[:],
            )
            nc.vector.tensor_copy(dots[:, 2 * L : 2 * L + 1], qdL[:])
            qqL = stats_pool.tile([P, 1], f32, tag="qqL")
            jaL = junk_pool.tile([P, D], f32, tag="junk_act")
            nc.scalar.activation(
                out=jaL[:], in_=qL[:], func=Act.Square, accum_out=qqL[:],
            )
            nc.vector.tensor_mul(nprod[:, 2 * L : 2 * L + 1], qqL[:], ddL[:])
            # Tile L's q-column epilogue runs early too; only the
            # a-column (2L+1) stays on the tail.
            cq = slice(2 * L, 2 * L + 1)
            nc.scalar.activation(inv[:, cq], nprod[:, cq], Act.Ln)
            nc.scalar.activation(inv[:, cq], inv[:, cq], Act.Exp, scale=0.5)
            nc.vector.tensor_scalar_max(inv[:, cq], inv[:, cq], EPS)
            nc.vector.reciprocal(inv[:, cq], inv[:, cq])
            nc.vector.tensor_mul(contrib[:, cq], dots[:, cq], inv[:, cq])
            nc.vector.tensor_mul(contrib[:, cq], contrib[:, cq], dvb[:, cq])

            for t in range(T - 1):
                # One contiguous [128, 12288] slab per tile: 48 KiB
                # per-partition rows keep the 16 HWDGE engines
                # bandwidth-limited instead of descriptor-limited.  The
                s_t = data_pool.tile([P, W], f32, tag="s")
                nc.sync.dma_start(s_t[:], flat[bass.ts(t, P), :])
                q = s_t[:, 0:D]
                a = s_t[:, D : 2 * D]
                d = s_t[:, 2 * D : 3 * D]

                # DVE: fused product + per-partition accumulate
                # (scalar_tensor_tensor; accum_out must be a standalone
                # tile — strided accum destinations crash the HW).
                dve_accs = {}
                for src0, src1, col, atag in (
                    (d, d, None, "dd1"),
                    (q, d, 2 * t, "qd1"),
                    (a, d, 2 * t + 1, "ad1"),
                ):
                    jd = junk_pool.tile([P, D], f32, tag="junk_dve")
                    acc = junk_pool.tile([P, 1], f32, tag=atag)
                    nc.vector.scalar_tensor_tensor(
                        out=jd[:], in0=src0, scalar=1.0, in1=src1,
                        op0=Alu.mult, op1=Alu.mult, accum_out=acc[:],
                    )
                    dve_accs[atag] = acc
                    if col is not None:
                        nc.vector.tensor_copy(dots[:, col : col + 1], acc[:])

                # ACT: square + accumulate for the q/a norms, then the
                # norm products (qq*dd, aa*dd) land in this tile's
                # columns.
                for src0, col, atag in ((q, 2 * t, "qq1"), (a, 2 * t + 1, "aa1")):
                    ja = junk_pool.tile([P, D], f32, tag="junk_act")
                    acc = junk_pool.tile([P, 1], f32, tag=atag)
                    nc.scalar.activation(
                        out=ja[:], in_=src0, func=Act.Square, accum_out=acc[:],
                    )
                    nc.vector.tensor_mul(
                        nprod[:, col : col + 1], acc[:], dve_accs["dd1"][:]
                    )

                # Per-tile epilogue on the contiguous [P, 2] slice —
                # hidden under the next tile's DMA.
                # cos = dot / max(sqrt(nprod), EPS), with
                # sqrt(v) = exp(0.5*ln(v)) so the whole kernel stays on
                # the natural_log_exp activation table (no reload).
                c2 = slice(2 * t, 2 * t + 2)
                nc.scalar.activation(inv[:, c2], nprod[:, c2], Act.Ln)
                nc.scalar.activation(inv[:, c2], inv[:, c2], Act.Exp, scale=0.5)
                nc.vector.tensor_scalar_max(inv[:, c2], inv[:, c2], EPS)
                nc.vector.reciprocal(inv[:, c2], inv[:, c2])
                nc.vector.tensor_mul(contrib[:, c2], dots[:, c2], inv[:, c2])
                nc.vector.tensor_mul(contrib[:, c2], contrib[:, c2], dvb[:, c2])

            # --- Tile L's a arrives last (split in half); only <a,d>,
            # |a|^2 and the final [P,1] epilogue remain on the tail.
            aL = stats_pool.tile([P, D], f32, tag="aL")
            nc.sync.dma_start(aL[:, 0:h], samples[bass.ts(L, P), 1, 0:h])
            nc.sync.dma_start(aL[:, h:D], samples[bass.ts(L, P), 1, h:D])

            # Everything but the last tile's a-column is final here; dump
            # it early so only 512 B of output DMA remains on the tail.
            nc.sync.dma_start(out[:, 0 : 2 * T - 1], contrib[:, 0 : 2 * T - 1])

            jd = junk_pool.tile([P, D], f32, tag="junk_dve")
            adA = junk_pool.tile([P, 1], f32, tag="ad1")
            adB = junk_pool.tile([P, 1], f32, tag="ad1b")
            nc.vector.scalar_tensor_tensor(
                out=jd[:, 0:h], in0=aL[:, 0:h], scalar=1.0, in1=dL[:, 0:h],
                op0=Alu.mult, op1=Alu.mult, accum_out=adA[:],
            )
            nc.vector.scalar_tensor_tensor(
                out=jd[:, h:D], in0=aL[:, h:D], scalar=1.0, in1=dL[:, h:D],
                op0=Alu.mult, op1=Alu.mult, accum_out=adB[:],
            )
            acol = 2 * L + 1
            ca = slice(acol, acol + 1)
            nc.vector.tensor_add(dots[:, ca], adA[:], adB[:])

            ja2 = junk_pool.tile([P, D], f32, tag="junk_act")
            aaA = junk_pool.tile([P, 1], f32, tag="aa1")
            aaB = junk_pool.tile([P, 1], f32, tag="aa1b")
            aa_sum = junk_pool.tile([P, 1], f32, tag="aa_sum")
            nc.scalar.activation(
                out=ja2[:, 0:h], in_=aL[:, 0:h], func=Act.Square,
                accum_out=aaA[:],
            )
            nc.scalar.activation(
                out=ja2[:, h:D], in_=aL[:, h:D], func=Act.Square,
                accum_out=aaB[:],
            )
            nc.vector.tensor_add(aa_sum[:], aaA[:], aaB[:])
            nc.vector.tensor_mul(nprod[:, ca], aa_sum[:], ddL[:])

            nc.scalar.activation(inv[:, ca], nprod[:, ca], Act.Ln)
            nc.scalar.activation(inv[:, ca], inv[:, ca], Act.Exp, scale=0.5)
            nc.vector.tensor_scalar_max(inv[:, ca], inv[:, ca], EPS)
            nc.vector.reciprocal(inv[:, ca], inv[:, ca])
            nc.vector.tensor_mul(contrib[:, ca], dots[:, ca], inv[:, ca])
            nc.vector.tensor_mul(contrib[:, ca], contrib[:, ca], dvb[:, ca])

            nc.sync.dma_start(out[:, 2 * T - 1 : 2 * T], contrib[:, ca])

    nc.compile()
    return nc


def _get_program():
    if "nc" not in _CACHE:
        _CACHE["nc"] = _build_program()
    return _CACHE["nc"]


def kernel(samples, labels, D_v1, D_v2):
    samples = np.asarray(samples, dtype=np.float32)
    labels = np.asarray(labels, dtype=np.float32)
    D_v1 = np.asarray(D_v1, dtype=np.float32)
    D_v2 = np.asarray(D_v2, dtype=np.float32)
    assert samples.shape == (B, 3, D), samples.shape

    nc = _get_program()

    in_maps = []
    for c in range(N_CORES):
        sl = slice(c * BS, (c + 1) * BS)
        in_maps.append(
            {
                "samples": np.ascontiguousarray(samples[sl]),
                "dv1": np.ascontiguousarray(D_v1[sl]),
                "dv2": np.ascontiguousarray(D_v2[sl]),
            }
        )

    _tc = os.environ.get("KERNEL_TRACE_CORES")
    _kw = {"trace_cores": [int(x) for x in _tc.split(",")]} if _tc else {}
    try:
        res = bass_utils.run_bass_kernel_spmd(
            nc, in_maps, core_ids=list(range(N_CORES)), **_kw
        )
    except Exception:
        # A previously-wedged NeuronCore surfaces as an unrecoverable
        # exec error on the first attempt; the runtime resets it, so a
        # single retry recovers.
        res = bass_utils.run_bass_kernel_spmd(
            nc, in_maps, core_ids=list(range(N_CORES)), **_kw
        )
    _CACHE["last_results"] = res

    # Gather/unshard: sum the per-core per-sample contributions to the
    # scalar score, then the 13-flop BCE epilogue.
    s = 0.0
    for c in range(N_CORES):
        s += float(np.sum(np.asarray(res.results[c]["out"], dtype=np.float64)))
    y = float(labels.reshape(-1)[0])
    bce = max(s, 0.0) - s * y + np.log1p(np.exp(-abs(s)))
    return np.asarray(bce, dtype=np.float32).reshape(())
